# revision 2
# baseline (speedup 1.0000x reference)
"""DeepSeek MoE on 8 trn2 cores — sparse expert dispatch (Bass/Tile).

Expert parallelism with on-device routing + token compaction. Each core owns
4 routed experts (host-balanced: 2 with >128 routed tokens, 2 with <=128) and
a 256-wide slice of the shared experts.

Device pipeline per core:
  1. exact fp32 logits + grouped top-k routing (combine weights cw_all)
  2. per-local-expert token compaction (gpsimd sparse_gather) -> slot lists
  3. ap_gather: gather routed tokens' hidden vectors into transposed tiles
  4. token-stationary matmuls: stationary = gathered token tile [k,128],
     moving = expert weights; silu*up*cw; PE transpose; down-projection
  5. shared experts computed densely (token-stationary)
  6. outputs: routed slot slab [768, H] bf16 + slot->token ids + shared [T, H]
Host: unshard = sum shared partials + scatter-add routed slabs by exported ids.
"""

import sys

sys.path.insert(0, "/opt/trn_rl_repo")

import numpy as np
import ml_dtypes

import concourse.bass as bass  # noqa: F401
import concourse.mybir as mybir
import concourse.tile as tile
from concourse import bacc
from concourse.bass_utils import run_bass_kernel_spmd
from concourse.masks import make_identity

F32 = mybir.dt.float32
BF16 = mybir.dt.bfloat16
I16 = mybir.dt.int16
I32 = mybir.dt.int32
AF = mybir.ActivationFunctionType
ALU = mybir.AluOpType

T = 512
H = 2048
I = 1024
E = 32
K = 8
NG = 8
TG = 4
SCALE = 2.5
NCORES = 8
EL = 4                    # local experts per core
SI = 256                  # shared-intermediate slice per core
P = 128
HK = H // P               # 16
KP = HK // 2              # 8 hidden k-pairs (for ap_gather d=2 packing)
TM = T // P               # 4
IK = I // P               # 8
NEG1 = -1.0e30
NEG2 = -2.0e30

# capacity config: slots per local expert (sorted desc), total slot count
CAPS = (256, 256, 128, 128)
CTOT = sum(CAPS)          # 768
NST = CTOT // P           # 6 stationary token tiles
SGF = 44                  # sparse_gather input free size (512 data + 192 pad)


def _routing(tc, d, pools, xT32):
    """cw_all [128, TM, E]: dense combine weights, exact fp32 (baseline)."""
    nc = tc.nc
    sb, work, stream, wstore, psB, psGU, psDN = pools

    gwT = sb.tile([P, HK, E], F32, name="gwT")
    nc.sync.dma_start(gwT[:], d["gwT"].rearrange("(k p) e -> p k e", p=P))
    gbb = sb.tile([P, E], F32, name="gbb")
    nc.sync.dma_start(gbb[:], d["gbb"][:])
    neg = sb.tile([P, E], F32, name="neg")
    nc.vector.memset(neg[:], NEG1)

    plgT = psB.tile([E, T], F32, tag="small")
    for k in range(HK):
        nc.tensor.matmul(plgT[:], gwT[:, k, :], xT32[:, k, :],
                         start=(k == 0), stop=(k == HK - 1))
    lgT = work.tile([E, T], F32, tag="lgT")
    nc.vector.tensor_copy(lgT[:], plgT[:])

    ident = sb.tile([P, P], F32, name="ident")
    make_identity(nc, ident)

    cw_all = sb.tile([P, TM, E], F32, name="cw_all")
    for m in range(TM):
        plg = psB.tile([P, E], F32, tag="small")
        nc.tensor.transpose(plg[:], lgT[:, m * P:(m + 1) * P], ident[:E, :E])
        s_t = work.tile([P, E], F32, tag="s_t")
        nc.scalar.activation(s_t[:], plg[:], AF.Sigmoid)
        sc = work.tile([P, E], F32, tag="sc")
        nc.vector.tensor_add(sc[:], s_t[:], gbb[:])

        sc3 = sc[:].rearrange("p (g f) -> p g f", f=4)
        ga = work.tile([P, NG], F32, tag="ga")
        gb_ = work.tile([P, NG], F32, tag="gb_")
        gc = work.tile([P, NG], F32, tag="gc")
        gd = work.tile([P, NG], F32, tag="gd")
        nc.vector.tensor_tensor(ga[:], sc3[:, :, 0], sc3[:, :, 1], ALU.max)
        nc.vector.tensor_tensor(gb_[:], sc3[:, :, 0], sc3[:, :, 1], ALU.min)
        nc.vector.tensor_tensor(gc[:], sc3[:, :, 2], sc3[:, :, 3], ALU.max)
        nc.vector.tensor_tensor(gd[:], sc3[:, :, 2], sc3[:, :, 3], ALU.min)
        hi = work.tile([P, NG], F32, tag="hi")
        lo = work.tile([P, NG], F32, tag="lo")
        mid = work.tile([P, NG], F32, tag="mid")
        nc.vector.tensor_tensor(hi[:], ga[:], gc[:], ALU.max)
        nc.vector.tensor_tensor(lo[:], ga[:], gc[:], ALU.min)
        nc.vector.tensor_tensor(mid[:], gb_[:], gd[:], ALU.max)
        gsc = work.tile([P, NG], F32, tag="gsc")
        nc.vector.tensor_tensor(gsc[:], lo[:], mid[:], ALU.max)
        nc.vector.tensor_add(gsc[:], gsc[:], hi[:])

        gm8 = work.tile([P, 8], F32, tag="gm8")
        nc.vector.max(gm8[:], gsc[:])
        nc.vector.memset(gm8[:, TG:], NEG1)
        gz = work.tile([P, NG], F32, tag="gz")
        nc.vector.match_replace(out=gz[:], in_to_replace=gm8[:], in_values=gsc[:], imm_value=NEG1)
        gmask = work.tile([P, NG], mybir.dt.uint32, tag="gmask")
        nc.vector.tensor_scalar(gmask[:], gz[:], -5.0e29, None, op0=ALU.is_le)

        emask = work.tile([P, E], mybir.dt.uint32, tag="emask")
        em3 = emask[:].rearrange("p (g f) -> p g f", f=4)
        nc.vector.tensor_copy(em3[:], gmask[:, :, None].to_broadcast([P, NG, 4]))
        msk = work.tile([P, E], F32, tag="msk")
        nc.vector.select(out=msk[:], mask=emask[:], on_true=sc[:], on_false=neg[:])

        t8 = work.tile([P, 8], F32, tag="t8")
        nc.vector.max(t8[:], msk[:])
        mz = work.tile([P, E], F32, tag="mz")
        nc.vector.match_replace(out=mz[:], in_to_replace=t8[:], in_values=msk[:], imm_value=NEG2)
        sel = work.tile([P, E], F32, tag="selm")
        nc.vector.tensor_scalar(sel[:], mz[:], -1.5e30, None, op0=ALU.is_le)

        wr = work.tile([P, E], F32, tag="wr")
        nc.vector.tensor_mul(wr[:], s_t[:], sel[:])
        ws = work.tile([P, 1], F32, tag="ws")
        nc.vector.reduce_sum(ws[:], wr[:], axis=mybir.AxisListType.X)
        rec = work.tile([P, 1], F32, tag="rec")
        nc.vector.reciprocal(rec[:], ws[:])
        coef = work.tile([P, 1], F32, tag="coef")
        nc.vector.tensor_scalar_mul(coef[:], rec[:], SCALE)
        nc.vector.tensor_scalar_mul(cw_all[:, m, :], wr[:], coef[:])
    return cw_all, ident


def _dispatch(tc, d, pools, cw_all, ident, xTi, xg, idxr):
    """Compact routed tokens per local expert.

    Returns idxr [128, CTOT//16] int16 (replicated slot->token ids) and
    cwb128 [128, NST] f32 (per-slot combine weight, slot j at [j%128, j//128]).
    """
    nc = tc.nc
    sb, work, stream, wstore, psB, psGU, psDN = pools

    # packed[tok, e] = sel ? 8*tok + cw : -1 ; then transpose to [E, T]
    packedT = sb.tile([E, T], F32, name="packedT")
    for m in range(TM):
        pk = work.tile([P, E], F32, tag="pk")
        msk = work.tile([P, E], mybir.dt.uint32, tag="pkm")
        nc.vector.tensor_scalar(msk[:], cw_all[:, m, :], 0.0, None, op0=ALU.is_gt)
        base = work.tile([P, E], F32, tag="pkb")
        nc.vector.tensor_scalar_add(base[:], cw_all[:, m, :], d_iota8(tc, d, m))
        neg = work.tile([P, E], F32, tag="pkn")
        nc.vector.memset(neg[:], -1.0)
        nc.vector.select(out=pk[:], mask=msk[:], on_true=base[:], on_false=neg[:])
        ptp = psB.tile([E, P], F32, tag="small")
        nc.tensor.transpose(ptp[:], pk[:], ident[:])
        nc.vector.tensor_copy(packedT[:, m * P:(m + 1) * P], ptp[:])

    # localize expert j via one-hot matmul, then DRAM-roundtrip to [16, 32]
    bselS = sb.tile([E, EL * P], F32, name="bselS")
    nc.sync.dma_start(bselS[:], d["bsel"][:])
    comp = sb.tile([16, CTOT // 16], F32, name="comp")
    idx16 = sb.tile([16, CTOT // 16], I16, name="idx16")
    idfa = sb.tile([16, CTOT // 16], F32, name="idfa")
    cwsl = sb.tile([16, CTOT // 16], F32, name="cwsl")
    off = 0
    for j in range(EL):
        pb = psB.tile([P, T], F32, tag="small")
        nc.tensor.matmul(pb[:], bselS[:, j * P:(j + 1) * P], packedT[:],
                         start=True, stop=True)
        row = work.tile([1, T], F32, tag="row")
        nc.vector.tensor_copy(row[:], pb[0:1, :])
        nc.sync.dma_start(d["pscr"][j], row[:])
        sgin = work.tile([16, SGF], F32, tag="sgin", name=f"sgin{j}")
        nc.vector.memset(sgin[:, 32:], 0.0)
        nc.sync.dma_start(sgin[:, 0:32],
                          d["pscr"][j].rearrange("(f p) -> p f", p=16))
        cj = CAPS[j] // 16
        nf = work.tile([1, 1], mybir.dt.uint32, tag="nf", name=f"nf{j}")
        nc.gpsimd.sparse_gather(comp[:, off:off + cj], sgin[:], num_found=nf[:])
        # decode: id = round(comp/8) (exact floor since cw<4); cw = comp-8*id
        idq = work.tile([16, cj], F32, tag="idq", name=f"idq{j}")
        nc.vector.tensor_scalar_mul(idq[:], comp[:, off:off + cj], 0.125)
        idi = work.tile([16, cj], I32, tag="idi", name=f"idi{j}")
        nc.vector.tensor_copy(idi[:], idq[:])
        nc.vector.tensor_copy(idx16[:, off:off + cj], idi[:])
        nc.vector.tensor_copy(idfa[:, off:off + cj], idi[:])
        t8n = work.tile([16, cj], F32, tag="t8n", name=f"t8n{j}")
        nc.vector.tensor_scalar_mul(t8n[:], idfa[:, off:off + cj], -8.0)
        nc.vector.tensor_add(cwsl[:, off:off + cj], comp[:, off:off + cj], t8n[:])
        for r in range(8):
            nc.sync.dma_start(
                idxr[:].rearrange("(r p) f -> r p f", p=16)[r, :, off:off + cj],
                idx16[:, off:off + cj])
        nc.gpsimd.ap_gather(xg[:, off * 16:(off + cj) * 16, :], xTi[:], idxr[:, off:off + cj],
                            channels=P, num_elems=T, d=HK, num_idxs=cj * 16)
        off += cj

    # export slot ids for host combine
    nc.sync.dma_start(d["outIdx"][:], idfa[:])
    # cw per slot -> [128, NST] via DRAM roundtrip: slot j=(j%16,j//16) -> (j%128,j//128)
    nc.sync.dma_start(d["cscr"].rearrange("c (g p) -> p (c g)", p=16), cwsl[:])
    cwb128 = sb.tile([P, NST], F32, name="cwb128")
    nc.sync.dma_start(cwb128[:], d["cscr"].rearrange("c p -> p c"))
    return cwb128


_IOTA_CACHE = {}


def d_iota8(tc, d, m):
    nc = tc.nc
    if "t" not in _IOTA_CACHE:
        sb = _IOTA_CACHE["sb"]
        t = sb.tile([P, TM], F32, name="iota8")
        nc.sync.dma_start(t[:], d["iota8"][:])
        _IOTA_CACHE["t"] = t
    return _IOTA_CACHE["t"][:, m:m + 1]


def _build_body(tc, d, pools):
    nc = tc.nc
    sb, work, stream, wstore, psB, psGU, psDN = pools
    _IOTA_CACHE.clear()
    _IOTA_CACHE["sb"] = sb

    # x loads: fp32 transposed (logits) + bf16 interleaved pairs (gather/shared)
    xT32 = sb.tile([P, HK, T], F32, name="xT32")
    xr32 = d["xT"].rearrange("(k p) t -> p k t", p=P)
    for k in range(HK):
        nc.sync.dma_start(xT32[:, k, :], xr32[:, k, :])
    xTi = sb.tile([P, T, HK], BF16, name="xTi")
    nc.sync.dma_start(xTi[:], d["xTi"][:])

    cw_all, ident = _routing(tc, d, pools, xT32)
    identB = sb.tile([P, P], BF16, name="identB")
    nc.vector.tensor_copy(identB[:], ident[:])
    # xg[p, slot, b] = x[token_slot, b*128 + p]  (filled per expert in _dispatch)
    xg = sb.tile([P, CTOT, HK], BF16, name="xg")
    idxr = sb.tile([P, CTOT // 16], I16, name="idxr")
    cwb128 = _dispatch(tc, d, pools, cw_all, ident, xTi, xg, idxr)

    # ---------------- shared experts (dense, token-stationary) ----------------
    swg = sb.tile([P, HK, 512], BF16, name="swg")
    nc.sync.dma_start(swg[:], d["swgu"][:])
    swdt = sb.tile([P, 2, 4, 512], BF16, name="swdt")
    nc.sync.dma_start(swdt[:], d["swd"][:])
    for m in range(TM):
        psg = psGU.tile([P, 512], F32, tag="gu", name=f"shg{m}")
        for k in range(HK):
            nc.tensor.matmul(psg[:], xTi[:, m * P:(m + 1) * P, k],
                             swg[:, k, :], start=(k == 0), stop=(k == HK - 1))
        sgm = work.tile([P, SI], F32, tag="sgm")
        nc.scalar.activation(sgm[:], psg[:, 0:SI], AF.Sigmoid)
        nc.vector.tensor_mul(sgm[:], sgm[:], psg[:, 0:SI])
        acts = work.tile([P, SI], BF16, tag="acts")
        nc.vector.tensor_mul(acts[:], sgm[:], psg[:, SI:2 * SI])
        actTs = work.tile([P, 2, P], BF16, tag="actTs")
        for t in range(2):
            ptr = psGU.tile([P, P], BF16, tag="gu", name=f"shtr{m}{t}")
            nc.tensor.transpose(ptr[:], acts[:, t * P:(t + 1) * P], identB[:])
            nc.vector.tensor_copy(actTs[:, t, :], ptr[:])
        ob = work.tile([P, H], BF16, tag="shob")
        for hh in range(2):
            psd = psDN.tile([P, 1024], F32, tag="dn", name=f"shd{m}{hh}")
            for i2 in range(2):
                for g in range(2):
                    nc.tensor.matmul(psd[:, g * 512:(g + 1) * 512],
                                     actTs[:, i2, :], swdt[:, i2, 2 * hh + g, :],
                                     start=(i2 == 0), stop=(i2 == 1))
            nc.vector.tensor_copy(ob[:, hh * 1024:(hh + 1) * 1024], psd[:])
        nc.sync.dma_start(
            d["outSh"].rearrange("(m p) h -> p m h", p=P)[:, m, :], ob[:])

    # ---------------- routed experts (sparse, token-stationary) --------------
    # group slot tiles by expert so each expert's weights stream exactly once
    etiles = []              # per expert: list of slot-tile indices
    off = 0
    for j in range(EL):
        etiles.append([off // P + s for s in range(CAPS[j] // P)])
        off += CAPS[j]

    for j in range(EL):
        tiles = etiles[j]
        nt = len(tiles)
        # gate_up, halves outer: per half h, psums = (gate_h, up_h) x tiles
        acts = {st: work.tile([P, I], BF16, tag=f"act{st % 2}", name=f"act{j}_{st}")
                for st in tiles}
        for h in range(2):
            pgs = {st: [psGU.tile([P, 512], F32, tag="gu", name=f"gu{j}{h}{st}{g}")
                        for g in range(2)] for st in tiles}
            for k in range(HK):
                wt = stream.tile([P, 2, 512], BF16, tag="wgu")
                nc.sync.dma_start(wt[:], d["wgu"][j, h, k])
                for st in tiles:
                    stat = xg[:, st * P:(st + 1) * P, k]
                    for g in range(2):
                        nc.tensor.matmul(pgs[st][g][:], stat, wt[:, g, :],
                                         start=(k == 0), stop=(k == HK - 1))
            for st in tiles:
                sgm = work.tile([P, 512], F32, tag="rsg")
                nc.scalar.activation(sgm[:], pgs[st][0][:], AF.Sigmoid)
                nc.vector.tensor_mul(sgm[:], sgm[:], pgs[st][0][:])
                nc.vector.tensor_mul(sgm[:], sgm[:], pgs[st][1][:])
                nc.vector.tensor_scalar_mul(acts[st][:, h * 512:(h + 1) * 512],
                                            sgm[:], cwb128[:, st:st + 1])
        actTs = {}
        for st in tiles:
            actT = work.tile([P, IK, P], BF16, tag=f"actT{st % 2}", name=f"actT{j}_{st}")
            for t in range(IK):
                ptr = psGU.tile([P, P], BF16, tag="gu", name=f"rtr{st}{t}")
                nc.tensor.transpose(ptr[:], acts[st][:, t * P:(t + 1) * P], identB[:])
                nc.vector.tensor_copy(actT[:, t, :], ptr[:])
            actTs[st] = actT
        wdc = wstore.tile([P, IK, 4, 512], BF16, tag="wdc")
        nc.sync.dma_start(wdc[:], d["wd"][j])
        for st in tiles:
            oq = work.tile([P, H], BF16, tag="oq")
            for hh in range(2):
                psd = psDN.tile([P, 1024], F32, tag="dn", name=f"dn{st}{hh}")
                for i2 in range(IK):
                    for g in range(2):
                        nc.tensor.matmul(psd[:, g * 512:(g + 1) * 512],
                                         actTs[st][:, i2, :], wdc[:, i2, 2 * hh + g, :],
                                         start=(i2 == 0), stop=(i2 == IK - 1))
                nc.vector.tensor_copy(oq[:, hh * 1024:(hh + 1) * 1024], psd[:])
            nc.sync.dma_start(
                d["outR"].rearrange("(s p) h -> p s h", p=P)[:, st, :], oq[:])


def build_nc(repeat=1):
    nc = bacc.Bacc("TRN2", target_bir_lowering=False, debug=False, num_devices=NCORES)
    d = {
        "xT": nc.dram_tensor("xT", [H, T], F32, kind="ExternalInput").ap(),
        "xTi": nc.dram_tensor("xTi", [P, T, HK], BF16, kind="ExternalInput").ap(),
        "gwT": nc.dram_tensor("gwT", [H, E], F32, kind="ExternalInput").ap(),
        "gbb": nc.dram_tensor("gbb", [P, E], F32, kind="ExternalInput").ap(),
        "bsel": nc.dram_tensor("bsel", [E, EL * P], F32, kind="ExternalInput").ap(),
        "iota8": nc.dram_tensor("iota8", [P, TM], F32, kind="ExternalInput").ap(),
        "wgu": nc.dram_tensor("wgu", [EL, 2, HK, P, 2, 512], BF16, kind="ExternalInput").ap(),
        "wd": nc.dram_tensor("wd", [EL, P, IK, 4, 512], BF16, kind="ExternalInput").ap(),
        "swgu": nc.dram_tensor("swgu", [P, HK, 512], BF16, kind="ExternalInput").ap(),
        "swd": nc.dram_tensor("swd", [P, 2, 4, 512], BF16, kind="ExternalInput").ap(),
        "pscr": nc.dram_tensor("pscr", [EL, T], F32, kind="Internal").ap(),
        "cscr": nc.dram_tensor("cscr", [NST, P], F32, kind="Internal").ap(),
        "outR": nc.dram_tensor("outR", [CTOT, H], BF16, kind="ExternalOutput").ap(),
        "outSh": nc.dram_tensor("outSh", [T, H], BF16, kind="ExternalOutput").ap(),
        "outIdx": nc.dram_tensor("outIdx", [16, CTOT // 16], F32, kind="ExternalOutput").ap(),
    }
    with tile.TileContext(nc) as tc:
        with (
            tc.tile_pool(name="sb", bufs=1) as sb,
            tc.tile_pool(name="work", bufs=2) as work,
            tc.tile_pool(name="stream", bufs=4) as stream,
            tc.tile_pool(name="wstore", bufs=1) as wstore,
            tc.tile_pool(name="psB", bufs=2, space="PSUM") as psB,
            tc.tile_pool(name="psGU", bufs=4, space="PSUM") as psGU,
            tc.tile_pool(name="psDN", bufs=1, space="PSUM") as psDN,
        ):
            pools = (sb, work, stream, wstore, psB, psGU, psDN)
            if repeat == 1:
                _build_body(tc, d, pools)
            else:
                with tc.For_i(0, repeat, 1):
                    _build_body(tc, d, pools)
    nc.compile()
    return nc


def _route_host(x, gw, gb):
    """Exact routing in numpy (for expert->core assignment only)."""
    logits = (x.astype(np.float64) @ gw.T.astype(np.float64)).astype(np.float32)
    s = (1.0 / (1.0 + np.exp(-logits.astype(np.float64)))).astype(np.float32)
    sc = s + gb[None, :]
    grp = sc.reshape(T, NG, 4)
    top2 = np.sort(grp, -1)[:, :, 2:].sum(-1)
    gidx = np.argsort(top2, -1, kind="stable")[:, -TG:]
    gmask = np.zeros((T, NG), bool)
    np.put_along_axis(gmask, gidx, True, -1)
    emask = np.repeat(gmask, 4, 1)
    masked = np.where(emask, sc, -np.inf)
    ids = np.argsort(masked, -1, kind="stable")[:, -K:]
    return np.bincount(ids.ravel(), minlength=E)


def shard_inputs(hidden_states, gate_w, gate_bias, w_gate_up, w_down,
                 shared_w_gate_up, shared_w_down):
    bf = ml_dtypes.bfloat16
    x = np.ascontiguousarray(hidden_states, dtype=np.float32)
    gw = np.asarray(gate_w, np.float32)
    gb = np.asarray(gate_bias, np.float32)
    xT = np.ascontiguousarray(x.T)
    # xTi[p, t, b] = x[t, b*128 + p]
    xTi = np.ascontiguousarray(
        x.astype(bf).reshape(T, HK, P).transpose(2, 0, 1))
    gwT = np.ascontiguousarray(gw.T)
    gbb = np.ascontiguousarray(np.tile(gb[None, :], (P, 1)))
    iota8 = np.ascontiguousarray(
        (np.arange(T, dtype=np.float32) * 8.0).reshape(TM, P).T)
    wgu = np.asarray(w_gate_up, np.float32)
    wd = np.asarray(w_down, np.float32)
    swgu = np.asarray(shared_w_gate_up, np.float32)
    swd = np.asarray(shared_w_down, np.float32)

    counts = _route_host(x, gw, gb)
    order = np.argsort(-counts, kind="stable")
    big, small = order[:16], order[16:]
    assign = [[big[2 * c], big[2 * c + 1], small[2 * c], small[2 * c + 1]]
              for c in range(NCORES)]
    for c in range(NCORES):
        for j, e in enumerate(assign[c]):
            if counts[e] > CAPS[j]:
                raise RuntimeError(f"capacity overflow: expert {e} cnt {counts[e]}")

    in_maps = []
    for c in range(NCORES):
        exps = assign[c]
        bsel = np.zeros((E, EL * P), dtype=np.float32)
        for j, e in enumerate(exps):
            bsel[e, j * P:(j + 1) * P] = 1.0
        # wgu_r[j, h, k, p, g, c]: g=0 gate half h, g=1 up half h
        wgu_c = wgu[exps].astype(bf)
        wgu_r = np.ascontiguousarray(
            wgu_c.reshape(EL, HK, P, 4, 512)[:, :, :, [0, 2, 1, 3], :]
            .reshape(EL, HK, P, 2, 2, 512).transpose(0, 3, 1, 2, 4, 5))
        # wd_r[j, i2, p, hq, c] = wd[e][i2*128+p, hq*512+c]
        wd_c = wd[exps].astype(bf)
        wd_r = np.ascontiguousarray(wd_c.reshape(EL, IK, P, 4, 512).transpose(0, 2, 1, 3, 4))
        sw = np.concatenate([
            swgu[:, c * SI:(c + 1) * SI],
            swgu[:, 2 * I + c * SI: 2 * I + (c + 1) * SI],
        ], axis=1).astype(bf)
        swgu_r = np.ascontiguousarray(sw.reshape(HK, P, 512).transpose(1, 0, 2))
        sd = swd[c * SI:(c + 1) * SI, :].astype(bf)
        swd_r = np.ascontiguousarray(sd.reshape(2, P, 4, 512).transpose(1, 0, 2, 3))
        in_maps.append({
            "xT": xT, "xTi": xTi, "gwT": gwT, "gbb": gbb, "bsel": bsel,
            "iota8": iota8, "wgu": wgu_r, "wd": wd_r, "swgu": swgu_r,
            "swd": swd_r,
        })
    return in_maps


def combine(results):
    """Host unshard: shared partial sum + scatter-add routed slabs."""
    acc = np.zeros((T, H), dtype=np.float32)
    for c in range(NCORES):
        acc += np.asarray(results[c]["outSh"], np.float32)
    for c in range(NCORES):
        ids = np.asarray(results[c]["outIdx"], np.float32).T.ravel().astype(np.int64)
        slab = np.asarray(results[c]["outR"], np.float32)
        o = np.argsort(ids, kind="stable")
        sid, sslab = ids[o], slab[o]
        bnd = np.flatnonzero(np.r_[True, sid[1:] != sid[:-1]])
        red = np.add.reduceat(sslab, bnd, axis=0)
        acc[sid[bnd]] += red
    return acc


_NC_CACHE = {}


def kernel(hidden_states, gate_w, gate_bias, w_gate_up, w_down,
           shared_w_gate_up, shared_w_down):
    if "nc" not in _NC_CACHE:
        _NC_CACHE["nc"] = build_nc(repeat=1)
    nc = _NC_CACHE["nc"]
    in_maps = shard_inputs(hidden_states, gate_w, gate_bias, w_gate_up, w_down,
                           shared_w_gate_up, shared_w_down)
    res = run_bass_kernel_spmd(nc, in_maps, list(range(NCORES)))
    return np.ascontiguousarray(combine(res.results))
